# revision 1
# baseline (speedup 1.0000x reference)
"""Batch multi-head graph attention (GAT) kernel for 8 Trainium2 NeuronCores.

Reference computation (per batch b, head h; n=1024 nodes, f_in=128, f_out=64):
    hp      = h @ w[h]                              # [n, 64]
    t       = tanh(hp)
    src     = t @ a_src[h];  dst = t @ a_dst[h]     # [n]
    score   = leaky_relu(src[i] + dst[j], 0.2)
    attn    = softmax over j of score masked by adj[i, j] > 0
    out     = attn @ hp + bias

Kernel reformulation (exact, modulo fp):
    exp(leaky(x)) = max(exp(x), exp(0.2 x))   (exp monotone, leaky = max(x, .2x))
    with x = src_i + dst_j both branches are rank-1 separable. Dividing by the
    row constant exp(0.2 src_i) (cancels in softmax):
        m[j, i] = adjT[j, i] * max(P_i * H_j, F_j)
        P = exp(0.8 src), H = exp(dst), F = exp(0.2 dst)
    out[i, :] = (sum_j m[j,i] * hp[j,:]) / (sum_j m[j,i]) + bias
    The denominator comes free as a ones-column appended to hp; bias is added
    on the host.

Aggregation runs with the masked-score chunk as the matmul *stationary* and
the (f_out+1)-wide hp-augmented tile as the *moving* operand, producing the
output directly in natural [node, feature] layout (no transposes).

P/H/F all come from one column-family of matmuls (a3 = [.8 a_src, a_dst,
.2 a_dst]) followed by a single batched exp; P columns bounce through DRAM to
become a broadcast row.

Sharding: data-parallel over batch (16 -> 2 per core); params replicated.
"""

from contextlib import ExitStack

import numpy as np

import concourse.tile as tile
from concourse import bacc, mybir
from concourse._compat import with_exitstack
from concourse.bass_utils import run_bass_kernel_spmd

F32 = mybir.dt.float32
F16 = mybir.dt.float16
F32R = mybir.dt.float32r
AL = mybir.AluOpType
AF = mybir.ActivationFunctionType

N_CORES = 8
BS = 16
B_PER_CORE = BS // N_CORES  # 2
N = 1024
F_IN = 128
F_OUT = 64
N_HEAD = 4
NCH = N // 128  # 8 chunks of 128 nodes

# Per-unit count of leading j-chunks whose mask-mult runs on GpSimd (Pool);
# the rest run on DVE. Pool mult is ~3.6x slower per chunk but runs in
# parallel with DVE.
POOL_CNT = [3, 3, 2, 3, 2, 3, 2, 1]
# Per-unit count of trailing j-chunks whose score op runs on ACT as
# relu(P*H - F) (with the F part folded into the aggregation as an extra
# adj @ (F*haug) matmul family), offloading the DVE tensor_scalar.
ACT_CNT = [0, 0, 2, 2, 2, 2, 2, 3]
FAST_PATH = False
PREP1_EARLY = True
PHP_GATE_B1 = True
# Per-unit count of u-ops hoisted onto Pool (fills its late-kernel idle)
POOL_U = [0, 0, 0, 0, 0, 0, 0, 0]


@with_exitstack
def _gat_tile_kernel(ctx: ExitStack, tc: tile.TileContext, out_ap, hT_ap, adjT_ap,
                     w_ap, a3_ap):
    nc = tc.nc

    singles = ctx.enter_context(tc.tile_pool(name="singles", bufs=1))
    hT_pool = ctx.enter_context(tc.tile_pool(name="hT", bufs=2))
    af_pool = ctx.enter_context(tc.tile_pool(name="af", bufs=2))
    tT_pool = ctx.enter_context(tc.tile_pool(name="tT", bufs=8))
    hf_pool = ctx.enter_context(tc.tile_pool(name="hf", bufs=2))
    pcol_pool = ctx.enter_context(tc.tile_pool(name="pcol", bufs=2))
    pb_pool = ctx.enter_context(tc.tile_pool(name="pb", bufs=8))
    haug_pool = ctx.enter_context(tc.tile_pool(name="haug", bufs=2))
    u_pool = ctx.enter_context(tc.tile_pool(name="u", bufs=18))
    m_pool = ctx.enter_context(tc.tile_pool(name="m", bufs=26))
    small_pool = ctx.enter_context(tc.tile_pool(name="small", bufs=8))
    ot_pool = ctx.enter_context(tc.tile_pool(name="ot", bufs=2))

    dram_pool = ctx.enter_context(tc.tile_pool(name="dram", bufs=8, space="DRAM"))
    ps_hpT = ctx.enter_context(tc.tile_pool(name="ps_hpT", bufs=1, space="PSUM"))
    ps_php = ctx.enter_context(tc.tile_pool(name="ps_php", bufs=1, space="PSUM"))
    ps_prc = ctx.enter_context(tc.tile_pool(name="ps_prc", bufs=1, space="PSUM"))
    ps_agg = ctx.enter_context(tc.tile_pool(name="ps_agg", bufs=4, space="PSUM"))

    def emit_loads(b, params=False):
        hT_sb = hT_pool.tile([F_IN, N], F32R, tag="hT", name=f"hT{b}")
        # split halves so the first hp matmuls start sooner; w between the
        # halves (needed with hT0), a3 after (needed later, with hT1)
        nc.sync.dma_start(hT_sb[:, 0:512], hT_ap[b, :, 0:512])
        if params:
            w_sb = singles.tile([F_IN, N_HEAD, F_OUT], F32R)  # [f_in, h, o]
            nc.sync.dma_start(w_sb[:], w_ap)
        nc.sync.dma_start(hT_sb[:, 512:N], hT_ap[b, :, 512:N])
        if params:
            a3_sb = singles.tile([F_OUT, N_HEAD, 3], F32)  # [o,h,(.8s,d,.2d)]
            nc.sync.dma_start(a3_sb[:], a3_ap)
            emit_loads.w_sb, emit_loads.a3_sb = w_sb, a3_sb
        af = af_pool.tile([128, NCH, N], F16, tag="af", name=f"af{b}")
        return dict(hT_sb=hT_sb, af=af)

    def emit_af(b, af, jcs, gate=None, gate_eng=None):
        # two DMAs per j-chunk: 364ns transfer quanta let the
        # latency-critical pd/pb bounce DMAs slot into the shared DMA
        # engines with half the queueing delay
        for jc in jcs:
            for half in range(2):
                lo = half * 512
                if gate is not None:
                    # 1-elem overlap write: delays this DMA until `gate` is
                    # produced, keeping the DMA engines free early on
                    gate_eng.tensor_copy(af[0:1, jc:jc + 1, lo:lo + 1], gate)
                nc.sync.dma_start(
                    af[:, jc:jc + 1, lo:lo + 512],
                    adjT_ap[b, jc * 128:(jc + 1) * 128,
                            lo:lo + 512].rearrange("(c p) i -> p c i", p=128))

    def emit_prep(b, ld, hi=True):
        hT_sb = ld["hT_sb"]
        w_sb, a3_sb = emit_loads.w_sb, emit_loads.a3_sb

        hp_ctx = tc.high_priority() if hi else None
        if hp_ctx:
            hp_ctx.__enter__()
        # ---- per-head chain: hpT -> tanh -> projections -> exp -> pb ----
        # Chained per head so h0's pb broadcast (the gate for the first
        # u-op) completes as early as possible.
        tTs = [tT_pool.tile([F_OUT, N], F32, tag="tT", name=f"tT{b}{h}")
               for h in range(N_HEAD)]
        prc = ps_prc.tile([128, NCH, N_HEAD, 3], F32, tag="prc", name=f"prc{b}")
        hf = hf_pool.tile([128, NCH, N_HEAD, 2], F32, tag="hf", name=f"hf{b}")
        pcol = pcol_pool.tile([128, NCH, N_HEAD, 1], F16, tag="pcol",
                              name=f"pcol{b}")
        # ---- hp natural + ones column: haug[j, jc, h, 0:64]=hp, [..,64]=1 ----
        haug = haug_pool.tile([128, NCH, N_HEAD, F_OUT + 1], F16, tag="haug",
                              name=f"haug{b}")
        nc.gpsimd.memset(haug[:, :, :, F_OUT:F_OUT + 1], 1.0)

        def emit_php(q):
            # pair of node-chunks per PSUM bank; one copy per pair
            php = ps_php.tile([128, 2, N_HEAD, F_OUT], F32, tag="php",
                              name=f"php{b}{q}")
            if b == 0 or PHP_GATE_B1:
                # WAW gate on this head's tanh: stops the scheduler from
                # hoisting these matmuls ahead of the critical prc3+exp chain
                nc.vector.tensor_copy(php[0:1, 0, 0, 0:1], tTs[q][0:1, 0:1])
            for s in range(2):
                ic = 2 * q + s
                nc.tensor.matmul(php[:, s, :, :].rearrange("p h o -> p (h o)"),
                                 hT_sb[:, ic * 128:(ic + 1) * 128],
                                 w_sb[:].rearrange("f h o -> f (h o)"),
                                 start=True, stop=True)
            # GPSIMD cannot read PSUM: b0 copies run on DVE (idle during the
            # ramp); b1's on ACT (slack mid-kernel)
            if b == 0:
                nc.vector.tensor_copy(haug[:, 2 * q:2 * q + 2, :, 0:F_OUT],
                                      php[:])
            else:
                nc.scalar.copy(haug[:, 2 * q:2 * q + 2, :, 0:F_OUT], php[:])

        negF = small_pool.tile([128, NCH, N_HEAD], F32, tag="negF",
                               name=f"negF{b}")
        pbs = []
        for h in range(N_HEAD):
            p = ps_hpT.tile([F_OUT, N], F32, tag="p", name=f"p{b}{h}")
            for c in range(2):
                nc.tensor.matmul(p[:, c * 512:(c + 1) * 512], w_sb[:, h, :],
                                 hT_sb[:, c * 512:(c + 1) * 512],
                                 start=True, stop=True)
            nc.scalar.activation(tTs[h][:], p[:], AF.Tanh)
            for jc in range(NCH):
                nc.tensor.matmul(prc[:, jc, h, :],
                                 tTs[h][:, jc * 128:(jc + 1) * 128],
                                 a3_sb[:, h, :], start=True, stop=True)
            pb = pb_pool.tile([128, N], F16, tag="pb", name=f"pb{b}{h}")
            if FAST_PATH and b == 0 and h == 0:
                # Fast path for the very first unit: src-row matmul, exp,
                # then a rank-1 ones-broadcast matmul + DVE copies. Avoids
                # the two high-latency DRAM bounce DMAs on the critical
                # path to the first u-op. PSUM comes from the ps_agg ring,
                # which is idle until the first unit's aggregation.
                nc.scalar.activation(hf[:, :, h, :], prc[:, :, h, 1:3],
                                     AF.Exp)
                prow = small_pool.tile([1, N], F16, tag="prow", name="prow")
                for c in range(2):
                    rr = ps_agg.tile([1, 512], F32, tag="pq", name=f"rr{c}")
                    nc.tensor.matmul(rr[:], a3_sb[:, 0, 0:1],
                                     tTs[0][:, c * 512:(c + 1) * 512],
                                     start=True, stop=True)
                    nc.scalar.activation(prow[0:1, c * 512:(c + 1) * 512],
                                         rr[:], AF.Exp)
                for c in range(2):
                    bc = ps_agg.tile([128, 512], F32, tag="pq", name=f"bc{c}")
                    nc.tensor.matmul(bc[:], ones16[0:1, :],
                                     prow[0:1, c * 512:(c + 1) * 512],
                                     start=True, stop=True)
                    nc.vector.tensor_copy(pb[:, c * 512:(c + 1) * 512], bc[:])
            else:
                nc.scalar.activation(pcol[:, :, h, :], prc[:, :, h, 0:1],
                                     AF.Exp)
                nc.scalar.activation(hf[:, :, h, :], prc[:, :, h, 1:3],
                                     AF.Exp)
                # P columns -> DRAM row -> broadcast to all partitions.
                # Issued from the SP queue (idle after the input loads) so
                # they don't serialize behind tanh/exp on the ACT queue.
                pd = dram_pool.tile([1, N], F16, tag="pd", name=f"pd{b}{h}")
                nc.sync.dma_start(pd.rearrange("u (c p) -> p c u", p=128),
                                  pcol[:, :, h, :])
                nc.sync.dma_start(pb[:], pd[0:1, :].to_broadcast([128, N]))
            pbs.append(pb)
            if sum(ACT_CNT) > 0:
                # negated F column for this head (ACT relu-path bias)
                nc.vector.tensor_scalar(negF[:, :, h], hf[:, :, h, 1], -1.0,
                                        None, AL.mult)
            # hp-natural matmuls interleave after each head's chain so the
            # scheduler cannot hoist them ahead of the critical prc3+exp
            emit_php(h)
        if hp_ctx:
            hp_ctx.__exit__(None, None, None)

        ot = ot_pool.tile([128, N_HEAD, NCH, F_OUT], F32, tag="ot", name=f"ot{b}")
        return dict(b=b, af=ld["af"], hf=hf, pbs=pbs, haug=haug, ot=ot,
                    pcol=pcol, negF=negF)

    def emit_scores(st, h, k, inject=None):
        # n^2 scores + fused aggregation for one (b, head)
        b, af, hf, haug = st["b"], st["af"], st["hf"], st["haug"]
        pb = st["pbs"][h]
        pq = [ps_agg.tile([128, 4, F_OUT + 1], F32, tag="pq", name=f"pq{b}{h}{q}")
              for q in range(2)]
        npool, nact, npu = POOL_CNT[k], ACT_CNT[k], POOL_U[k]
        # ACT-path score ops and F-restoration operands are hoisted to the
        # unit start: they depend only on prep outputs, so ACT/DVE produce
        # them in parallel with the unit's earlier chunks
        us, fas = {}, {}
        for jc in range(NCH - nact, NCH):
            u = u_pool.tile([128, N], F16, tag="u", name=f"u{b}{h}{jc}")
            nc.scalar.activation(u[:], pb[:], AF.Relu,
                                 scale=hf[:, jc, h, 0:1],
                                 bias=st["negF"][:, jc, h:h + 1])
            us[jc] = u
            fa = small_pool.tile([128, F_OUT + 1], F16, tag="fa",
                                 name=f"fa{b}{h}{jc}")
            nc.vector.tensor_scalar(fa[:], haug[:, jc, h, :],
                                    hf[:, jc, h, 1:2], None, AL.mult)
            fas[jc] = fa
        for jc in range(npool, npool + npu):
            u = u_pool.tile([128, N], F16, tag="u", name=f"u{b}{h}{jc}")
            nc.gpsimd.tensor_scalar(u[:], pb[:], hf[:, jc, h, 0:1],
                                    hf[:, jc, h, 1:2], AL.mult, AL.max)
            us[jc] = u
        for jc in range(NCH):
            if jc == 3 and inject is not None:
                emit_finalize(*inject, last=False)
            if jc in us:
                u = us[jc]
            else:
                u = u_pool.tile([128, N], F16, tag="u", name=f"u{b}{h}{jc}")
                nc.vector.tensor_scalar(u[:], pb[:], hf[:, jc, h, 0:1],
                                        hf[:, jc, h, 1:2], AL.mult, AL.max)
            m = m_pool.tile([128, N], F16, tag="m", name=f"m{b}{h}{jc}")
            # last unit: Pool (idle at kernel end) also takes the first
            # ACT-chunk's mask-mult, shortening the tail's DVE stream
            pool_jc = jc < npool or (k == len(POOL_CNT) - 1 and jc == NCH - nact)
            eng = nc.gpsimd if pool_jc else nc.vector
            eng.tensor_tensor(m[:], u[:], af[:, jc, :], AL.mult)
            # HW start=True zeroes the whole PSUM bank for the written
            # partitions, so only the FIRST matmul per bank may carry it:
            # at jc0, write ic3/ic7 first (start), the rest accumulate onto
            # the zeroed bank.
            ics = [3, 0, 1, 2, 7, 4, 5, 6] if jc == 0 else range(NCH)
            for ic in ics:
                nc.tensor.matmul(pq[ic // 4][:, ic % 4, :],
                                 m[:, ic * 128:(ic + 1) * 128],
                                 haug[:, jc, h, :],
                                 start=(jc == 0 and ic % 4 == 3),
                                 stop=(jc == NCH - 1 and nact == 0),
                                 skip_group_check=True)
        # F-restoration family for ACT-path chunks closes the groups
        for i, jc in enumerate(range(NCH - nact, NCH)):
            fa = fas[jc]
            for ic in range(NCH):
                nc.tensor.matmul(pq[ic // 4][:, ic % 4, :],
                                 af[:, jc, ic * 128:(ic + 1) * 128], fa[:],
                                 start=False, stop=(i == nact - 1),
                                 skip_group_check=True)
        return pq

    def emit_finalize(st, h, pq, last):
        b = st["b"]
        rden = small_pool.tile([128, NCH], F32, tag="rden", name=f"rden{b}{h}")
        for q in range(2):
            nc.vector.reciprocal(rden[:, q * 4:(q + 1) * 4], pq[q][:, :, F_OUT])
        ot = st["ot"]
        if not last:
            for ic in range(NCH):
                nc.scalar.mul(ot[:, h, ic, :], pq[ic // 4][:, ic % 4, 0:F_OUT],
                              rden[:, ic:ic + 1])
            nc.scalar.dma_start(out_ap[b, h], ot[:, h])
        else:
            # tail: spread divisions across engines (Pool cannot read PSUM),
            # split the out DMA
            div_eng = [nc.scalar, nc.vector, nc.scalar, nc.vector] * 2
            for ic in range(NCH):
                eng = div_eng[ic]
                if eng is nc.scalar:
                    eng.mul(ot[:, h, ic, :], pq[ic // 4][:, ic % 4, 0:F_OUT],
                            rden[:, ic:ic + 1])
                else:
                    eng.tensor_scalar(ot[:, h, ic, :],
                                      pq[ic // 4][:, ic % 4, 0:F_OUT],
                                      rden[:, ic:ic + 1], None, AL.mult)
                if ic == 3:
                    nc.scalar.dma_start(out_ap[b, h, :, 0:4], ot[:, h, 0:4])
            nc.scalar.dma_start(out_ap[b, h, :, 4:NCH], ot[:, h, 4:NCH])

    # All but the first adjacency chunk are gated on b0's first P-exp (DVE
    # gate copies: DVE is idle during the ramp), and b1's chunks on b0's
    # last adjacency chunk (Pool copies), so the big loads never front-run
    # the latency-critical pb bounce DMAs on the shared DMA engines. b1's
    # hT load sits after b0's gated chunks in the SP queue, which delays it
    # the same way.
    # PE warmup: dummy matmuls from t~0.5 keep the tensor engine busy so the
    # first real matmuls run at full p-state instead of the cold-start rate
    warm = singles.tile([128, 260], F16)
    nc.gpsimd.memset(warm[:], 0.0)
    ones16 = singles.tile([1, 128], F16)
    nc.gpsimd.memset(ones16[:], 1.0)
    with tc.high_priority():
        for i in range(16):
            pw = ps_php.tile([128, 256], F32, tag="php", name=f"warm{i}")
            nc.tensor.matmul(pw[:], warm[:, 0:128], warm[:, 0:256],
                             start=True, stop=True)

    loads0 = emit_loads(0, params=True)
    emit_af(0, loads0["af"], range(0, 1))
    st0 = emit_prep(0, loads0)
    emit_af(0, loads0["af"], range(1, 4),
            gate=st0["hf"][0:1, 0:1, 0, 0:1], gate_eng=nc.vector)
    emit_af(0, loads0["af"], range(4, NCH),
            gate=st0["pbs"][0][0:1, 0:1], gate_eng=nc.vector)
    loads1 = emit_loads(1)
    emit_af(1, loads1["af"], range(NCH),
            gate=loads0["af"][0:1, NCH - 1:NCH, 0:1], gate_eng=nc.gpsimd)
    sts = [st0, None]
    if PREP1_EARLY:
        sts[1] = emit_prep(1, loads1, hi=False)
    # finalize for unit k is emitted after unit k+1's score stream so the
    # DVE reciprocal never head-of-line blocks the u/m pipeline. b1's prep
    # is emitted after unit 0 so none of its (waiting) ops sit ahead of the
    # early units in any engine queue.
    units = [(b, h) for b in range(B_PER_CORE) for h in range(N_HEAD)]
    pending = None  # (st, h, pq)
    for k, (b, h) in enumerate(units):
        if k == 1 and not PREP1_EARLY:
            sts[1] = emit_prep(1, loads1, hi=False)
        if k < len(units) - 1:
            pq = emit_scores(sts[b], h, k)
            if pending is not None:
                emit_finalize(*pending, last=False)
            pending = (sts[b], h, pq)
    # last unit: inject the previous unit's finalize after a few chunks so
    # its divisions/out-DMA don't all land in the tail
    b, h = units[-1]
    pq = emit_scores(sts[b], h, len(units) - 1, inject=pending)
    emit_finalize(sts[b], h, pq, last=True)


def _build_nc():
    nc = bacc.Bacc("TRN2", target_bir_lowering=False, debug=False,
                   num_devices=N_CORES)
    hT = nc.dram_tensor("hT", [B_PER_CORE, F_IN, N], F32R,
                        kind="ExternalInput").ap()
    adjT = nc.dram_tensor("adjT", [B_PER_CORE, N, N], F16,
                          kind="ExternalInput").ap()
    w = nc.dram_tensor("w", [F_IN, N_HEAD, F_OUT], F32R,
                       kind="ExternalInput").ap()
    a3 = nc.dram_tensor("a3", [F_OUT, N_HEAD, 3], F32,
                        kind="ExternalInput").ap()
    out = nc.dram_tensor("out", [B_PER_CORE, N_HEAD, 128, NCH, F_OUT], F32,
                         kind="ExternalOutput").ap()
    with tile.TileContext(nc) as tc:
        _gat_tile_kernel(tc, out, hT, adjT, w, a3)
    nc.compile()
    return nc


_NC_CACHE = []


def _get_nc():
    if not _NC_CACHE:
        _NC_CACHE.append(_build_nc())
    return _NC_CACHE[0]


def make_in_maps(h, adj, w, a_src, a_dst, bias):
    h = np.asarray(h, dtype=np.float32)
    adj = np.asarray(adj)
    w = np.asarray(w, dtype=np.float32)
    a_src = np.asarray(a_src, np.float32)[..., 0]  # [h, o]
    a_dst = np.asarray(a_dst, np.float32)[..., 0]
    hT = np.ascontiguousarray(h.transpose(0, 2, 1))          # [bs, f_in, n]
    adjT = np.ascontiguousarray(
        adj.astype(np.float16).transpose(0, 2, 1))           # [bs, j, i] {0,1}
    wT = np.ascontiguousarray(w.transpose(1, 0, 2))          # [f_in, h, o]
    a3 = np.ascontiguousarray(
        np.stack([0.8 * a_src, a_dst, 0.2 * a_dst], axis=2)  # [h, o, 3]
        .transpose(1, 0, 2))                                 # [o, h, 3]
    in_maps = []
    for c in range(N_CORES):
        sl = slice(B_PER_CORE * c, B_PER_CORE * (c + 1))
        in_maps.append({"hT": hT[sl], "adjT": adjT[sl], "w": wT, "a3": a3})
    return in_maps


def kernel(h, adj, w, a_src, a_dst, bias):
    nc = _get_nc()
    in_maps = make_in_maps(h, adj, w, a_src, a_dst, bias)
    res = run_bass_kernel_spmd(nc, in_maps, core_ids=list(range(N_CORES)))
    out = np.concatenate([res.results[c]["out"] for c in range(N_CORES)], axis=0)
    # device layout [b, h, p, ic, o] -> [b, h, ic*128+p, o]
    out = out.transpose(0, 1, 3, 2, 4).reshape(BS, N_HEAD, N, F_OUT)
    out = out + np.asarray(bias, np.float32)[None, None, None, :]
    return np.ascontiguousarray(out.astype(np.float32))



# revision 7
# speedup vs baseline: 1.0454x; 1.0454x over previous
"""Batch multi-head graph attention (GAT) kernel for 8 Trainium2 NeuronCores.

Reference computation (per batch b, head h; n=1024 nodes, f_in=128, f_out=64):
    hp      = h @ w[h]                              # [n, 64]
    t       = tanh(hp)
    src     = t @ a_src[h];  dst = t @ a_dst[h]     # [n]
    score   = leaky_relu(src[i] + dst[j], 0.2)
    attn    = softmax over j of score masked by adj[i, j] > 0
    out     = attn @ hp + bias

Kernel reformulation (exact, modulo fp):
    exp(leaky(x)) = max(exp(x), exp(0.2 x))   (exp monotone, leaky = max(x, .2x))
    with x = src_i + dst_j both branches are rank-1 separable. Dividing by the
    row constant exp(0.2 src_i) (cancels in softmax):
        m[j, i] = adjT[j, i] * max(P_i * H_j, F_j)
        P = exp(0.8 src), H = exp(dst), F = exp(0.2 dst)
    out[i, :] = (sum_j m[j,i] * hp[j,:]) / (sum_j m[j,i]) + bias
    The denominator comes free as a ones-column appended to hp; bias is added
    on the host.

Aggregation runs with the masked-score chunk as the matmul *stationary* and
the (f_out+1)-wide hp-augmented tile as the *moving* operand, producing the
output directly in natural [node, feature] layout (no transposes).

P/H/F all come from one column-family of matmuls (a3 = [.8 a_src, a_dst,
.2 a_dst]) followed by a single batched exp; P columns bounce through DRAM to
become a broadcast row.

Sharding: data-parallel over batch (16 -> 2 per core); params replicated.
"""

from contextlib import ExitStack

import numpy as np

import concourse.tile as tile
from concourse import bacc, mybir
from concourse._compat import with_exitstack
from concourse.bass_utils import run_bass_kernel_spmd

F32 = mybir.dt.float32
F16 = mybir.dt.float16
F32R = mybir.dt.float32r
AL = mybir.AluOpType
AF = mybir.ActivationFunctionType

N_CORES = 8
BS = 16
B_PER_CORE = BS // N_CORES  # 2
N = 1024
F_IN = 128
F_OUT = 64
N_HEAD = 4
NCH = N // 128  # 8 chunks of 128 nodes

# Per-unit count of leading j-chunks whose mask-mult runs on GpSimd (Pool);
# the rest run on DVE. Pool mult is ~3.6x slower per chunk but runs in
# parallel with DVE.
POOL_CNT = [3, 3, 2, 3, 2, 3, 2, 1]
# Per-unit count of trailing j-chunks whose score op runs on ACT as
# relu(P*H - F) (with the F part folded into the aggregation as an extra
# adj @ (F*haug) matmul family), offloading the DVE tensor_scalar.
ACT_CNT = [0, 0, 2, 2, 2, 2, 2, 3]
FAST_PATH = False
PREP1_EARLY = True
PHP_GATE_B1 = True
# Per-unit count of u-ops hoisted onto Pool (fills its late-kernel idle)
POOL_U = [0, 0, 0, 0, 0, 0, 0, 0]
SCALE_OUT = 2.0 ** -8


@with_exitstack
def _gat_tile_kernel(ctx: ExitStack, tc: tile.TileContext, out_ap, hT_ap, adjT_ap,
                     w_ap, a3_ap):
    nc = tc.nc

    singles = ctx.enter_context(tc.tile_pool(name="singles", bufs=1))
    hT_pool = ctx.enter_context(tc.tile_pool(name="hT", bufs=2))
    af_pool = ctx.enter_context(tc.tile_pool(name="af", bufs=2))
    tT_pool = ctx.enter_context(tc.tile_pool(name="tT", bufs=8))
    hf_pool = ctx.enter_context(tc.tile_pool(name="hf", bufs=2))
    pcol_pool = ctx.enter_context(tc.tile_pool(name="pcol", bufs=2))
    pb_pool = ctx.enter_context(tc.tile_pool(name="pb", bufs=8))
    haug_pool = ctx.enter_context(tc.tile_pool(name="haug", bufs=2))
    u_pool = ctx.enter_context(tc.tile_pool(name="u", bufs=18))
    m_pool = ctx.enter_context(tc.tile_pool(name="m", bufs=26))
    small_pool = ctx.enter_context(tc.tile_pool(name="small", bufs=8))
    ot_pool = ctx.enter_context(tc.tile_pool(name="ot", bufs=2))

    dram_pool = ctx.enter_context(tc.tile_pool(name="dram", bufs=8, space="DRAM"))
    ps_hpT = ctx.enter_context(tc.tile_pool(name="ps_hpT", bufs=1, space="PSUM"))
    ps_php = ctx.enter_context(tc.tile_pool(name="ps_php", bufs=1, space="PSUM"))
    ps_prc = ctx.enter_context(tc.tile_pool(name="ps_prc", bufs=1, space="PSUM"))
    ps_agg = ctx.enter_context(tc.tile_pool(name="ps_agg", bufs=4, space="PSUM"))

    def emit_loads(b, params=False):
        hT_sb = hT_pool.tile([F_IN, N], F32R, tag="hT", name=f"hT{b}")
        # split halves so the first hp matmuls start sooner; w between the
        # halves (needed with hT0), a3 after (needed later, with hT1)
        nc.sync.dma_start(hT_sb[:, 0:512], hT_ap[b, :, 0:512])
        if params:
            w_sb = singles.tile([F_IN, N_HEAD, F_OUT], F32R)  # [f_in, h, o]
            nc.sync.dma_start(w_sb[:], w_ap)
        nc.sync.dma_start(hT_sb[:, 512:N], hT_ap[b, :, 512:N])
        if params:
            a3_sb = singles.tile([F_OUT, N_HEAD, 3], F32)  # [o,h,(.8s,d,.2d)]
            nc.sync.dma_start(a3_sb[:], a3_ap)
            emit_loads.w_sb, emit_loads.a3_sb = w_sb, a3_sb
        af = af_pool.tile([128, NCH, N], F16, tag="af", name=f"af{b}")
        return dict(hT_sb=hT_sb, af=af)

    def emit_af(b, af, jcs, gate=None, gate_eng=None):
        # two DMAs per j-chunk: 364ns transfer quanta let the
        # latency-critical pd/pb bounce DMAs slot into the shared DMA
        # engines with half the queueing delay
        for jc in jcs:
            for half in range(2):
                lo = half * 512
                if gate is not None:
                    # 1-elem overlap write: delays this DMA until `gate` is
                    # produced, keeping the DMA engines free early on
                    gate_eng.tensor_copy(af[0:1, jc:jc + 1, lo:lo + 1], gate)
                nc.sync.dma_start(
                    af[:, jc:jc + 1, lo:lo + 512],
                    adjT_ap[b, jc * 128:(jc + 1) * 128,
                            lo:lo + 512].rearrange("(c p) i -> p c i", p=128))

    def emit_prep(b, ld, hi=True):
        hT_sb = ld["hT_sb"]
        w_sb, a3_sb = emit_loads.w_sb, emit_loads.a3_sb

        hp_ctx = tc.high_priority() if hi else None
        if hp_ctx:
            hp_ctx.__enter__()
        # ---- per-head chain: hpT -> tanh -> projections -> exp -> pb ----
        # Chained per head so h0's pb broadcast (the gate for the first
        # u-op) completes as early as possible.
        tTs = [tT_pool.tile([F_OUT, N], F32, tag="tT", name=f"tT{b}{h}")
               for h in range(N_HEAD)]
        prc = ps_prc.tile([128, NCH, N_HEAD, 3], F32, tag="prc", name=f"prc{b}")
        hf = hf_pool.tile([128, NCH, N_HEAD, 2], F32, tag="hf", name=f"hf{b}")
        pcol = pcol_pool.tile([128, NCH, N_HEAD, 1], F16, tag="pcol",
                              name=f"pcol{b}")
        # ---- hp natural + ones column: haug[j, jc, h, 0:64]=hp, [..,64]=1 ----
        haug = haug_pool.tile([128, NCH, N_HEAD, F_OUT + 1], F16, tag="haug",
                              name=f"haug{b}")
        nc.gpsimd.memset(haug[:, :, :, F_OUT:F_OUT + 1], 1.0)

        def emit_php(q):
            # pair of node-chunks per PSUM bank; one copy per pair
            php = ps_php.tile([128, 2, N_HEAD, F_OUT], F32, tag="php",
                              name=f"php{b}{q}")
            if b == 0 or PHP_GATE_B1:
                # WAW gate on this head's tanh: stops the scheduler from
                # hoisting these matmuls ahead of the critical prc3+exp chain
                nc.vector.tensor_copy(php[0:1, 0, 0, 0:1], tTs[q][0:1, 0:1])
            for s in range(2):
                ic = 2 * q + s
                nc.tensor.matmul(php[:, s, :, :].rearrange("p h o -> p (h o)"),
                                 hT_sb[:, ic * 128:(ic + 1) * 128],
                                 w_sb[:].rearrange("f h o -> f (h o)"),
                                 start=True, stop=True)
            # GPSIMD cannot read PSUM: b0 copies run on DVE (idle during the
            # ramp); b1's on ACT (slack mid-kernel)
            if b == 0:
                nc.vector.tensor_copy(haug[:, 2 * q:2 * q + 2, :, 0:F_OUT],
                                      php[:])
            else:
                nc.scalar.copy(haug[:, 2 * q:2 * q + 2, :, 0:F_OUT], php[:])

        negF = small_pool.tile([128, NCH, N_HEAD], F32, tag="negF",
                               name=f"negF{b}")
        pbs = []
        for h in range(N_HEAD):
            p = ps_hpT.tile([F_OUT, N], F32, tag="p", name=f"p{b}{h}")
            for c in range(2):
                nc.tensor.matmul(p[:, c * 512:(c + 1) * 512], w_sb[:, h, :],
                                 hT_sb[:, c * 512:(c + 1) * 512],
                                 start=True, stop=True)
            nc.scalar.activation(tTs[h][:], p[:], AF.Tanh)
            for jc in range(NCH):
                nc.tensor.matmul(prc[:, jc, h, :],
                                 tTs[h][:, jc * 128:(jc + 1) * 128],
                                 a3_sb[:, h, :], start=True, stop=True)
            pb = pb_pool.tile([128, N], F16, tag="pb", name=f"pb{b}{h}")
            if FAST_PATH and b == 0 and h == 0:
                # Fast path for the very first unit: src-row matmul, exp,
                # then a rank-1 ones-broadcast matmul + DVE copies. Avoids
                # the two high-latency DRAM bounce DMAs on the critical
                # path to the first u-op. PSUM comes from the ps_agg ring,
                # which is idle until the first unit's aggregation.
                nc.scalar.activation(hf[:, :, h, :], prc[:, :, h, 1:3],
                                     AF.Exp)
                prow = small_pool.tile([1, N], F16, tag="prow", name="prow")
                for c in range(2):
                    rr = ps_agg.tile([1, 512], F32, tag="pq", name=f"rr{c}")
                    nc.tensor.matmul(rr[:], a3_sb[:, 0, 0:1],
                                     tTs[0][:, c * 512:(c + 1) * 512],
                                     start=True, stop=True)
                    nc.scalar.activation(prow[0:1, c * 512:(c + 1) * 512],
                                         rr[:], AF.Exp)
                for c in range(2):
                    bc = ps_agg.tile([128, 512], F32, tag="pq", name=f"bc{c}")
                    nc.tensor.matmul(bc[:], ones16[0:1, :],
                                     prow[0:1, c * 512:(c + 1) * 512],
                                     start=True, stop=True)
                    nc.vector.tensor_copy(pb[:, c * 512:(c + 1) * 512], bc[:])
            else:
                nc.scalar.activation(pcol[:, :, h, :], prc[:, :, h, 0:1],
                                     AF.Exp)
                nc.scalar.activation(hf[:, :, h, :], prc[:, :, h, 1:3],
                                     AF.Exp)
                # P columns -> DRAM row -> broadcast to all partitions.
                # Issued from the SP queue (idle after the input loads) so
                # they don't serialize behind tanh/exp on the ACT queue.
                pd = dram_pool.tile([1, N], F16, tag="pd", name=f"pd{b}{h}")
                nc.sync.dma_start(pd.rearrange("u (c p) -> p c u", p=128),
                                  pcol[:, :, h, :])
                nc.sync.dma_start(pb[:], pd[0:1, :].to_broadcast([128, N]))
            pbs.append(pb)
            if sum(ACT_CNT) > 0:
                # negated F column for this head (ACT relu-path bias)
                nc.vector.tensor_scalar(negF[:, :, h], hf[:, :, h, 1], -1.0,
                                        None, AL.mult)
            # hp-natural matmuls interleave after each head's chain so the
            # scheduler cannot hoist them ahead of the critical prc3+exp
            emit_php(h)
        if hp_ctx:
            hp_ctx.__exit__(None, None, None)

        ot = ot_pool.tile([128, N_HEAD, 2, 4, F_OUT + 1], F16, tag="ot",
                          name=f"ot{b}")
        return dict(b=b, af=ld["af"], hf=hf, pbs=pbs, haug=haug, ot=ot,
                    pcol=pcol, negF=negF)

    def emit_scores(st, h, k, inject=None):
        # n^2 scores + fused aggregation for one (b, head)
        b, af, hf, haug = st["b"], st["af"], st["hf"], st["haug"]
        pb = st["pbs"][h]
        pq = [ps_agg.tile([128, 4, F_OUT + 1], F32, tag="pq", name=f"pq{b}{h}{q}")
              for q in range(2)]
        npool, nact, npu = POOL_CNT[k], ACT_CNT[k], POOL_U[k]
        # ACT-path score ops and F-restoration operands are hoisted to the
        # unit start: they depend only on prep outputs, so ACT/DVE produce
        # them in parallel with the unit's earlier chunks
        us, fas = {}, {}
        for jc in range(NCH - nact, NCH):
            u = u_pool.tile([128, N], F16, tag="u", name=f"u{b}{h}{jc}")
            nc.scalar.activation(u[:], pb[:], AF.Relu,
                                 scale=hf[:, jc, h, 0:1],
                                 bias=st["negF"][:, jc, h:h + 1])
            us[jc] = u
            fa = small_pool.tile([128, F_OUT + 1], F16, tag="fa",
                                 name=f"fa{b}{h}{jc}")
            nc.vector.tensor_scalar(fa[:], haug[:, jc, h, :],
                                    hf[:, jc, h, 1:2], None, AL.mult)
            fas[jc] = fa
        for jc in range(npool, npool + npu):
            u = u_pool.tile([128, N], F16, tag="u", name=f"u{b}{h}{jc}")
            nc.gpsimd.tensor_scalar(u[:], pb[:], hf[:, jc, h, 0:1],
                                    hf[:, jc, h, 1:2], AL.mult, AL.max)
            us[jc] = u
        for jc in range(NCH):
            if jc == 3 and inject is not None:
                emit_finalize(*inject, last=False)
            if jc in us:
                u = us[jc]
            else:
                u = u_pool.tile([128, N], F16, tag="u", name=f"u{b}{h}{jc}")
                nc.vector.tensor_scalar(u[:], pb[:], hf[:, jc, h, 0:1],
                                        hf[:, jc, h, 1:2], AL.mult, AL.max)
            m = m_pool.tile([128, N], F16, tag="m", name=f"m{b}{h}{jc}")
            # last unit: Pool (idle at kernel end) also takes the first
            # ACT-chunk's mask-mult, shortening the tail's DVE stream
            pool_jc = jc < npool or (k == len(POOL_CNT) - 1 and jc == NCH - nact)
            if pool_jc:
                # TensorScalarPtr opcode runs at 0.60 gpsimd efficiency vs
                # TensorTensor's 0.42 -> 1517ns instead of 2127ns per chunk
                nc.gpsimd.scalar_tensor_tensor(m[:], u[:], 1.0, af[:, jc, :],
                                               AL.bypass, AL.mult)
            else:
                nc.vector.tensor_tensor(m[:], u[:], af[:, jc, :], AL.mult)
            # HW start=True zeroes the whole PSUM bank for the written
            # partitions, so only the FIRST matmul per bank may carry it:
            # at jc0, write ic3/ic7 first (start), the rest accumulate onto
            # the zeroed bank.
            ics = [3, 0, 1, 2, 7, 4, 5, 6] if jc == 0 else range(NCH)
            for ic in ics:
                nc.tensor.matmul(pq[ic // 4][:, ic % 4, :],
                                 m[:, ic * 128:(ic + 1) * 128],
                                 haug[:, jc, h, :],
                                 start=(jc == 0 and ic % 4 == 3),
                                 stop=(jc == NCH - 1 and nact == 0),
                                 skip_group_check=True)
        # F-restoration family for ACT-path chunks closes the groups
        for i, jc in enumerate(range(NCH - nact, NCH)):
            fa = fas[jc]
            for ic in range(NCH):
                nc.tensor.matmul(pq[ic // 4][:, ic % 4, :],
                                 af[:, jc, ic * 128:(ic + 1) * 128], fa[:],
                                 start=False, stop=(i == nact - 1),
                                 skip_group_check=True)
        return pq

    def emit_finalize(st, h, pq, last):
        # Coarse PSUM->SBUF copy (one per pq bank, scaled by 2^-8 against f16
        # overflow) + f16 out DMA; the num/den division happens on the host.
        b = st["b"]
        ot = st["ot"]
        for q in range(2):
            eng = nc.vector if (last and q == 1) else nc.scalar
            if eng is nc.scalar:
                eng.activation(ot[:, h, q], pq[q][:], AF.Copy, scale=SCALE_OUT)
            else:
                eng.tensor_scalar(ot[:, h, q], pq[q][:], SCALE_OUT, None,
                                  AL.mult)
            nc.scalar.dma_start(out_ap[b, h, q], ot[:, h, q])

    # All but the first adjacency chunk are gated on b0's first P-exp (DVE
    # gate copies: DVE is idle during the ramp), and b1's chunks on b0's
    # last adjacency chunk (Pool copies), so the big loads never front-run
    # the latency-critical pb bounce DMAs on the shared DMA engines. b1's
    # hT load sits after b0's gated chunks in the SP queue, which delays it
    # the same way.
    # PE warmup: dummy matmuls from t~0.5 keep the tensor engine busy so the
    # first real matmuls run at full p-state instead of the cold-start rate
    warm = singles.tile([128, 260], F16)
    nc.gpsimd.memset(warm[:], 0.0)
    ones16 = singles.tile([1, 128], F16)
    nc.gpsimd.memset(ones16[:], 1.0)
    with tc.high_priority():
        for i in range(16):
            pw = ps_php.tile([128, 256], F32, tag="php", name=f"warm{i}")
            nc.tensor.matmul(pw[:], warm[:, 0:128], warm[:, 0:256],
                             start=True, stop=True)

    loads0 = emit_loads(0, params=True)
    emit_af(0, loads0["af"], range(0, 1))
    st0 = emit_prep(0, loads0)
    emit_af(0, loads0["af"], range(1, 4),
            gate=st0["hf"][0:1, 0:1, 0, 0:1], gate_eng=nc.vector)
    emit_af(0, loads0["af"], range(4, NCH),
            gate=st0["pbs"][0][0:1, 0:1], gate_eng=nc.vector)
    loads1 = emit_loads(1)
    emit_af(1, loads1["af"], range(NCH),
            gate=loads0["af"][0:1, NCH - 1:NCH, 0:1], gate_eng=nc.gpsimd)
    sts = [st0, None]
    if PREP1_EARLY:
        sts[1] = emit_prep(1, loads1, hi=False)
    # finalize for unit k is emitted after unit k+1's score stream so the
    # DVE reciprocal never head-of-line blocks the u/m pipeline. b1's prep
    # is emitted after unit 0 so none of its (waiting) ops sit ahead of the
    # early units in any engine queue.
    units = [(b, h) for b in range(B_PER_CORE) for h in range(N_HEAD)]
    pending = None  # (st, h, pq)
    for k, (b, h) in enumerate(units):
        if k == 1 and not PREP1_EARLY:
            sts[1] = emit_prep(1, loads1, hi=False)
        if k < len(units) - 1:
            pq = emit_scores(sts[b], h, k)
            if pending is not None:
                emit_finalize(*pending, last=False)
            pending = (sts[b], h, pq)
    # last unit: inject the previous unit's finalize after a few chunks so
    # its divisions/out-DMA don't all land in the tail
    b, h = units[-1]
    pq = emit_scores(sts[b], h, len(units) - 1, inject=pending)
    emit_finalize(sts[b], h, pq, last=True)


def _build_nc():
    nc = bacc.Bacc("TRN2", target_bir_lowering=False, debug=False,
                   num_devices=N_CORES)
    hT = nc.dram_tensor("hT", [B_PER_CORE, F_IN, N], F32R,
                        kind="ExternalInput").ap()
    adjT = nc.dram_tensor("adjT", [B_PER_CORE, N, N], F16,
                          kind="ExternalInput").ap()
    w = nc.dram_tensor("w", [F_IN, N_HEAD, F_OUT], F32R,
                       kind="ExternalInput").ap()
    a3 = nc.dram_tensor("a3", [F_OUT, N_HEAD, 3], F32,
                        kind="ExternalInput").ap()
    out = nc.dram_tensor("out", [B_PER_CORE, N_HEAD, 2, 128, 4, F_OUT + 1],
                         F16, kind="ExternalOutput").ap()
    with tile.TileContext(nc) as tc:
        _gat_tile_kernel(tc, out, hT, adjT, w, a3)
    nc.compile()
    return nc


_NC_CACHE = []


def _get_nc():
    if not _NC_CACHE:
        _NC_CACHE.append(_build_nc())
    return _NC_CACHE[0]


def make_in_maps(h, adj, w, a_src, a_dst, bias):
    h = np.asarray(h, dtype=np.float32)
    adj = np.asarray(adj)
    w = np.asarray(w, dtype=np.float32)
    a_src = np.asarray(a_src, np.float32)[..., 0]  # [h, o]
    a_dst = np.asarray(a_dst, np.float32)[..., 0]
    hT = np.ascontiguousarray(h.transpose(0, 2, 1))          # [bs, f_in, n]
    adjT = np.ascontiguousarray(
        adj.astype(np.float16).transpose(0, 2, 1))           # [bs, j, i] {0,1}
    wT = np.ascontiguousarray(w.transpose(1, 0, 2))          # [f_in, h, o]
    a3 = np.ascontiguousarray(
        np.stack([0.8 * a_src, a_dst, 0.2 * a_dst], axis=2)  # [h, o, 3]
        .transpose(1, 0, 2))                                 # [o, h, 3]
    in_maps = []
    for c in range(N_CORES):
        sl = slice(B_PER_CORE * c, B_PER_CORE * (c + 1))
        in_maps.append({"hT": hT[sl], "adjT": adjT[sl], "w": wT, "a3": a3})
    return in_maps


def kernel(h, adj, w, a_src, a_dst, bias):
    nc = _get_nc()
    in_maps = make_in_maps(h, adj, w, a_src, a_dst, bias)
    res = run_bass_kernel_spmd(nc, in_maps, core_ids=list(range(N_CORES)))
    out = np.concatenate([res.results[c]["out"] for c in range(N_CORES)], axis=0)
    # device layout [b, h, q, p, s, o+den]; node i = (q*4+s)*128 + p
    out = out.astype(np.float32).transpose(0, 1, 2, 4, 3, 5)
    out = out.reshape(BS, N_HEAD, N, F_OUT + 1)
    out = out[..., :F_OUT] / out[..., F_OUT:]
    out = out + np.asarray(bias, np.float32)[None, None, None, :]
    return np.ascontiguousarray(out.astype(np.float32))

